# revision 1
# baseline (speedup 1.0000x reference)
"""Trainium2 Bass kernel for nn_LocalReverseDiffusion.

Reference computation (per sample n):
  y[n,c,d*4+i,h*4+j,w*4+k] = x[n,c,d,h,w] * w_ct[c,i,j,k] + b_ct[c]
  yn = GroupNorm(1 group, affine gamma/beta) over (C,D,H,W) of y
  out[n,o,:,:,:] = sum_c w_pw[o,c] * yn[n,c,:,:,:]

Key identity used here: fold the whole chain into 64 small GEMMs (one per
conv-transpose offset (i,j,k)) applied to x directly:

  out[n,o,4d+i,4h+j,4w+k] = inv[n] * sum_c M0[ijk][o,c] * x[n,c,d,h,w] + C2[n,o]

  M0[ijk][o,c] = w_pw[o,c] * gamma[c] * w_ct[c,i,j,k]
  inv[n]       = rsqrt(var[n] + eps)
  C2[n,o]      = sum_c w_pw[o,c] * (gamma[c]*inv[n]*(b_ct[c]-mean[n]) + beta[c])

GroupNorm stats have a closed form in terms of per-(n,c) sums of x and x^2
(because the conv-transpose is a non-overlapping scatter), so they are
computed on-device from x with two reductions + a handful of tiny matmuls.

Sharding: 8 cores, core `cid` owns input depth planes {2cid, 2cid+1} ->
output slab out[:, :, 8cid:8cid+8, :, :] (16.8 MB of the 134 MB output).
Every core redundantly computes the full-sample stats from the full x
(2 MB) - cheaper and simpler than a cross-core all-reduce.
"""

import numpy as np

import concourse.bass as bass
import concourse.mybir as mybir
import concourse.tile as tile
from concourse import bacc
from concourse.bass_utils import run_bass_kernel_spmd

# Problem shape (hardcoded per harness contract)
N, C, D, H, W = 2, 64, 16, 16, 16
R = 4
NCORES = 8
DL = D // NCORES            # input d-planes per core = 2
DO_PER_CORE = DL * R        # output do-planes per core = 8
EPS = 1e-5
MT = float(C * D * H * W * R**3)   # elements per GroupNorm group = 16777216
PV = float(D * H * W * R**3)       # positions per channel = 262144

F32 = mybir.dt.float32
AF = mybir.ActivationFunctionType

_CACHE = {}


def _build_program(reps=1, no_out_dma=False):
    """Build the (single, SPMD) Bass program. Same program runs on all 8
    cores; per-core data differences come via the input tensors.

    reps>1 repeats the whole body (timing builds only): the wall-clock
    difference between reps=K and reps=1 isolates the device body time
    from per-execution runtime overhead."""
    nc = bacc.Bacc(
        "TRN2",
        target_bir_lowering=False,
        debug=False,
        enable_asserts=True,
        num_devices=NCORES,
    )

    # ---- DRAM I/O ----
    xs_d = nc.dram_tensor("xs", [N, C, DL, H, W], F32, kind="ExternalInput")
    xf_d = nc.dram_tensor("xf", [N * C, D * H * W], F32, kind="ExternalInput")
    lt_d = nc.dram_tensor("lt", [C, 4096], F32, kind="ExternalInput")
    sw_d = nc.dram_tensor("swall", [128, 1280], F32, kind="ExternalInput")
    gb_d = nc.dram_tensor("gb6", [C, 6], F32, kind="ExternalInput")
    wp_d = nc.dram_tensor("wpt2", [C, 128], F32, kind="ExternalInput")
    out_d = nc.dram_tensor(
        "out", [N, C, DO_PER_CORE, H * R, W * R], F32, kind="ExternalOutput"
    )

    with tile.TileContext(nc) as tc:
        with (
            tc.tile_pool(name="consts", bufs=1) as consts,
            tc.tile_pool(name="xfp", bufs=1) as xfp,
            tc.tile_pool(name="stats", bufs=1) as stats,
            tc.tile_pool(name="ot", bufs=2) as otp,
            tc.tile_pool(name="psum", bufs=4, space="PSUM") as psp,
            tc.tile_pool(name="psum_s", bufs=2, space="PSUM") as psp_s,
        ):
          for _rep in range(reps):
              # ---- ACT table warm-up (hide ~2.7us table loads under DMA) ----
              warm = stats.tile([128, 2], F32)
              nc.vector.memset(warm[:], 1.0)
              nc.scalar.sqrt(warm[:, 0:1], warm[:, 0:1])
              nc.scalar.square(warm[:, 0:1], warm[:, 0:1])
              nc.scalar.activation(warm[:, 0:1], warm[:, 0:1], AF.Identity, bias=warm[:, 1:2], scale=warm[:, 1:2])

              # ---- Load inputs ----
              xf_t = xfp.tile([128, 4096], F32)       # x as [(n c), dhw]
              nc.sync.dma_start(xf_t[:], xf_d.ap())
              xs_t = consts.tile([C, N * DL * H * W], F32)  # [c, (n dl h w)]
              nc.sync.dma_start(
                  xs_t[:].rearrange("c (n r) -> c n r", n=N),
                  xs_d.ap().rearrange("n c dl h w -> c n (dl h w)"),
              )
              lt_t = consts.tile([C, 4096], F32)
              nc.sync.dma_start(lt_t[:], lt_d.ap())
              sw_t = consts.tile([128, 1280], F32)
              nc.sync.dma_start(sw_t[:], sw_d.ap())
              gb_t = consts.tile([C, 6], F32)
              nc.sync.dma_start(gb_t[:], gb_d.ap())
              wp_t = consts.tile([C, 128], F32)
              nc.sync.dma_start(wp_t[:], wp_d.ap())

              # ---- Stats: per-(n,c) sum and sumsq of x ----
              P = stats.tile([128, 3], F32)
              nc.vector.memset(P[:, 2:3], 1.0)
              nc.vector.reduce_sum(P[:, 0:1], xf_t[:], axis=mybir.AxisListType.X)
              sq_t = xfp.tile([128, 4096], F32)
              nc.scalar.activation(
                  sq_t[:], xf_t[:], AF.Square, accum_out=P[:, 1:2]
              )

              # ---- Fold stats across channels (+ broadcast) via tiny matmuls.
              # swall blocks (each [128,128], all columns identical):
              #   b0: sw*n0   b1: sww*n0  b2: 2*b*sw*n0  b3: PV*b*n0  b4: PV*b^2*n0
              #   b5..b9: same masked for n1.
              # psum cols: 0 = M_tot*mean(n0), 1 = M_tot*E[y^2](n0), 2,3 = n1.
              ps_st = psp_s.tile([128, 4], F32)
              for nq in range(2):
                  o = 5 * nq
                  mc, ec = 2 * nq, 2 * nq + 1
                  nc.tensor.matmul(
                      ps_st[:, mc : mc + 1], sw_t[:, (o + 0) * 128 : (o + 1) * 128],
                      P[:, 0:1], start=True, stop=False,
                  )
                  nc.tensor.matmul(
                      ps_st[:, mc : mc + 1], sw_t[:, (o + 3) * 128 : (o + 4) * 128],
                      P[:, 2:3], start=False, stop=True,
                  )
                  nc.tensor.matmul(
                      ps_st[:, ec : ec + 1], sw_t[:, (o + 1) * 128 : (o + 2) * 128],
                      P[:, 1:2], start=True, stop=False,
                  )
                  nc.tensor.matmul(
                      ps_st[:, ec : ec + 1], sw_t[:, (o + 2) * 128 : (o + 3) * 128],
                      P[:, 0:1], start=False, stop=False,
                  )
                  nc.tensor.matmul(
                      ps_st[:, ec : ec + 1], sw_t[:, (o + 4) * 128 : (o + 5) * 128],
                      P[:, 2:3], start=False, stop=True,
                  )

              # mean/var/inv on all 128 partitions, cols = n
              mean_t = stats.tile([128, 2], F32)
              m2_t = stats.tile([128, 2], F32)
              for nq in range(2):
                  nc.vector.tensor_scalar_mul(
                      mean_t[:, nq : nq + 1], ps_st[:, 2 * nq : 2 * nq + 1], 1.0 / MT
                  )
                  nc.vector.tensor_scalar_mul(
                      m2_t[:, nq : nq + 1], ps_st[:, 2 * nq + 1 : 2 * nq + 2], 1.0 / MT
                  )
              msq_t = stats.tile([128, 2], F32)
              nc.vector.tensor_mul(msq_t[:], mean_t[:], mean_t[:])
              var_t = stats.tile([128, 2], F32)
              nc.vector.tensor_sub(var_t[:], m2_t[:], msq_t[:])
              nc.vector.tensor_scalar_add(var_t[:], var_t[:], EPS)
              rec_t = stats.tile([128, 2], F32)
              nc.vector.reciprocal(rec_t[:], var_t[:])
              inv_t = stats.tile([128, 2], F32)
              nc.scalar.sqrt(inv_t[:], rec_t[:])   # inv = sqrt(1/(var+eps))

              # ---- C2[n,o] = sum_c w_pw[o,c]*(gamma*inv*(b-mean)+beta) ----
              r2a = stats.tile([C, 2], F32)
              nc.vector.tensor_sub(r2a[:], gb_t[:, 2:4], mean_t[0:C, :])
              nc.vector.tensor_mul(r2a[:], r2a[:], inv_t[0:C, :])
              nc.vector.tensor_mul(r2a[:], r2a[:], gb_t[:, 0:2])
              nc.vector.tensor_add(r2a[:], r2a[:], gb_t[:, 4:6])
              ps_c2 = psp_s.tile([128, 2], F32)
              nc.tensor.matmul(ps_c2[:], wp_t[:], r2a[:], start=True, stop=True)
              c2_t = stats.tile([128, 2], F32)
              nc.vector.tensor_copy(c2_t[:], ps_c2[:])

              # ---- Main: 64 GEMMs + scaled scatter-copies + output DMA ----
              # lhsT layout: lt[:, pair*128 + g*64 + o] = M0[i=2*g+il, j, k][o, c]
              #   with pair = il*16 + j*4 + k.
              # psum partitions: p = g*64 + o.
              # OT cols: dl*8192 + il*4096 + hs*256 + j*64 + w*4 + k.
              # DRAM do-plane: do = 4*dl + 2*g + il -> per partition the two
              # il blocks are CONSECUTIVE do planes = 32KB contiguous DRAM.
              out_ap = out_d.ap().rearrange(
                  "n o (dl g il) ho wo -> n dl g o (il ho wo)", dl=DL, g=2, il=2
              )
              for n in range(N):
                  ot = otp.tile([128, 16384], F32, tag="ot")
                  ot_v = ot[:].rearrange(
                      "p (dl i2 hs j w k) -> p dl i2 hs j w k",
                      dl=DL, i2=2, hs=H, j=R, w=W, k=R,
                  )
                  for pair in range(32):
                      i2, j, k = pair // 16, (pair // 4) % 4, pair % 4
                      ps = psp.tile([128, 512], F32, tag="mm")
                      nc.tensor.matmul(
                          ps[:],
                          lt_t[:, pair * 128 : (pair + 1) * 128],
                          xs_t[:, n * 512 : (n + 1) * 512],
                          start=True, stop=True,
                      )
                      src = ps[:].rearrange("p (dl hs w) -> p dl hs w", dl=DL, hs=H, w=W)
                      dst = ot_v[:, :, i2, :, j, :, k]
                      if pair % 2 == 0:
                          nc.scalar.activation(
                              dst, src, AF.Identity,
                              bias=c2_t[:, n : n + 1], scale=inv_t[:, n : n + 1],
                          )
                      else:
                          nc.vector.tensor_scalar(
                              dst, src,
                              inv_t[:, n : n + 1], c2_t[:, n : n + 1],
                              op0=mybir.AluOpType.mult, op1=mybir.AluOpType.add,
                          )
                  if not no_out_dma:
                      for dl in range(DL):
                          nc.sync.dma_start(
                              out_ap[n, dl], ot[:, dl * 8192 : (dl + 1) * 8192]
                          )

    nc.compile()
    return nc


def _host_consts(w_ct, b_ct, gamma, beta, w_pw):
    w_ct = np.asarray(w_ct, np.float32).reshape(C, R, R, R)
    b_ct = np.asarray(b_ct, np.float32)
    gamma = np.asarray(gamma, np.float32)
    beta = np.asarray(beta, np.float32)
    w_pw = np.asarray(w_pw, np.float32).reshape(C, C)  # [o, c]

    gw = gamma[:, None, None, None] * w_ct  # [c, i, j, k]
    # LT6 [c, il, j, k, g, o]; i = 2*g + il  (psum half g owns do-planes
    # {4dl+2g, 4dl+2g+1} -> 32KB-contiguous DRAM writes per partition)
    sc_g0 = gw[:, 0:2]  # g=0: i = il in {0, 1}
    sc_g1 = gw[:, 2:4]  # g=1: i = 2+il
    sc = np.stack([sc_g0, sc_g1], axis=4)  # [c, il, j, k, g]
    lt = (sc[..., None] * w_pw.T[:, None, None, None, None, :]).reshape(C, 4096)
    lt = np.ascontiguousarray(lt, np.float32)

    wflat = w_ct.reshape(C, -1)
    sw = wflat.sum(1)
    sww = (wflat**2).sum(1)
    tbsw = 2.0 * b_ct * sw
    cb = PV * b_ct
    cb2 = PV * b_ct**2
    blocks = []
    for nq in range(2):
        for vec in (sw, sww, tbsw, cb, cb2):
            v = np.zeros(128, np.float32)
            v[nq * 64 : (nq + 1) * 64] = vec
            blocks.append(np.repeat(v[:, None], 128, axis=1))
    swall = np.ascontiguousarray(np.concatenate(blocks, axis=1), np.float32)

    gb6 = np.stack([gamma, gamma, b_ct, b_ct, beta, beta], axis=1)
    gb6 = np.ascontiguousarray(gb6, np.float32)
    wpt2 = np.ascontiguousarray(
        np.concatenate([w_pw.T, w_pw.T], axis=1), np.float32
    )
    return lt, swall, gb6, wpt2


def _get_nc(reps=1, no_out_dma=False):
    key = ("nc", reps, no_out_dma)
    if key not in _CACHE:
        _CACHE[key] = _build_program(reps, no_out_dma)
    return _CACHE[key]


def make_in_maps(x, w_ct, b_ct, gamma, beta, w_pw):
    x = np.ascontiguousarray(np.asarray(x, np.float32))
    lt, swall, gb6, wpt2 = _host_consts(w_ct, b_ct, gamma, beta, w_pw)
    xf = x.reshape(N * C, D * H * W)
    in_maps = []
    for cid in range(NCORES):
        xs = np.ascontiguousarray(x[:, :, 2 * cid : 2 * cid + 2])
        in_maps.append(
            dict(xs=xs, xf=xf, lt=lt, swall=swall, gb6=gb6, wpt2=wpt2)
        )
    return in_maps


def assemble(results):
    return np.concatenate(
        [results[cid]["out"] for cid in range(NCORES)], axis=2
    )


def kernel(x, w_ct, b_ct, gamma, beta, w_pw):
    nc = _get_nc()
    in_maps = make_in_maps(x, w_ct, b_ct, gamma, beta, w_pw)
    res = run_bass_kernel_spmd(nc, in_maps, list(range(NCORES))).results
    return assemble(res)

